# revision 1
# baseline (speedup 1.0000x reference)
"""Bass/Trainium2 kernel for nn_CrossSparseAttention.

Computes, for every (caption c, image i, word w):
    sims[c,i,w,r] = <caps[c,w], imgs[i,r]> / T   (masked by img/cap lengths)
    keep top-5 per row over r, masked softmax p, att = p @ imgs_m,
    out[i,c,w] = <att, caps[c,w]> / (||att|| + EPS), -1 where w >= cap_len.

Strategy (8 NeuronCores, caption axis sharded, imgs replicated):
  Phase A (rows = (c,w) on partitions, ragged (i, r<img_len) on free dim):
    fp32r matmuls give raw dots S; per-img max8 gives the top-8 of each
    (c,w,i) row in one DVE instruction -> rowmax m and threshold
    t' = (5th+6th)/2, both compact (P, 64).
  Phase B (flip layout: rows = (i,r) groups <= 128 partitions, (c,w) free):
    recompute S via fp32r matmul; subtract t' and m by folding small
    selector matmuls into the same PSUM accumulation; top-5 mask via
    saturated sigmoid on ACT; e = exp(10*(S-m)) * mask;
    B = e^T K e via host-precomputed per-img Gram block-diagonal matmul;
    per-img sums (s, A'=sum e*(S-m), B) via ones-matmuls accumulated into
    image-indexed stats PSUM across all groups.
  Final:  out = (A' + m*s) / (sqrt(B) + s*EPS)  on compact (64, NCW) tiles.
  Host scatters valid (c,w) columns into the full output and fills -1.

All algebra is exact w.r.t. the reference: with p = e/s,
  num = sum p*S = (A' + m*s)/s,  sq = B/s^2,  norm = sqrt(B)/s,
  out = num/(norm+EPS) = (A' + m*s)/(sqrt(B) + s*EPS).
"""

import numpy as np
import ml_dtypes
from contextlib import ExitStack

import concourse.bass as bass
import concourse.bacc as bacc
import concourse.tile as tile
import concourse.mybir as mybir
from concourse.bass_utils import run_bass_kernel_spmd

FP32 = mybir.dt.float32
FP32R = mybir.dt.float32r
BF16 = mybir.dt.bfloat16
ALU = mybir.AluOpType
ACTF = mybir.ActivationFunctionType

N_CORES = 8
N_IMG, R_PAD, D = 64, 36, 512
N_CAP, W_PAD = 64, 50
KNN = 5
INV_T = 10.0          # 1 / TEMPERATURE
EPS = -1e-8
MASK_VAL = -1.0
BIGSCALE = 1e12       # sigmoid(BIGSCALE * x) saturates to exact 0/1
KCHUNKS = 4           # 512 = 4 x 128 contraction chunks


def _pack(sizes, cap):
    """Greedy-pack consecutive items with sum(size) <= cap.
    Returns list of (start_item, end_item) (end exclusive)."""
    out = []
    s = 0
    while s < len(sizes):
        e = s
        tot = 0
        while e < len(sizes) and tot + sizes[e] <= cap:
            tot += sizes[e]
            e += 1
        out.append((s, e))
        s = e
    return out


def _build_program(lens, offs, NR, NCW, pchunks, groups, n_mt, mt_bounds,
                   debug_dump=False):
    """Build the SPMD bass program. All shape metadata is host-known."""
    nc = bacc.Bacc("TRN2", target_bir_lowering=False, debug=False)

    d_imgsT = nc.dram_tensor("imgsT", [D, NR], FP32R, kind="ExternalInput").ap()
    d_capsT = nc.dram_tensor("capsT", [D, NCW], FP32R, kind="ExternalInput").ap()
    d_imgsL = nc.dram_tensor("imgsL", [D, NR], FP32R, kind="ExternalInput").ap()
    d_capsL = nc.dram_tensor("capsL", [D, NCW], FP32R, kind="ExternalInput").ap()
    kbd_cols = sum(offs[e] - offs[s] for (s, e) in groups)
    d_kbd = nc.dram_tensor("kbd", [128, kbd_cols], FP32R, kind="ExternalInput").ap()
    esel_cols = kbd_cols
    d_esel = nc.dram_tensor("esel", [N_IMG, esel_cols], FP32R, kind="ExternalInput").ap()
    d_eselb = nc.dram_tensor("eselb", [N_IMG, esel_cols], BF16, kind="ExternalInput").ap()
    d_eselnb = nc.dram_tensor("eselnb", [N_IMG, esel_cols], BF16, kind="ExternalInput").ap()
    ones_cols = sum(ge for (_, ge) in groups)
    d_ones = nc.dram_tensor("onesbd", [128, ones_cols], FP32R, kind="ExternalInput").ap()
    d_ident = nc.dram_tensor("ident", [128, 128], FP32, kind="ExternalInput").ap()
    d_pbias = nc.dram_tensor("padbias", [128, max(1, len(groups))], FP32,
                             kind="ExternalInput").ap()
    d_out = nc.dram_tensor("out", [N_IMG, NCW], FP32, kind="ExternalOutput").ap()
    if debug_dump:
        d_dbg_S = nc.dram_tensor("dbg_S", [128, NR], FP32, kind="ExternalOutput").ap()
        d_dbg_mx = nc.dram_tensor("dbg_mx", [128, N_IMG * 8], FP32, kind="ExternalOutput").ap()
        d_dbg_r1 = nc.dram_tensor("dbg_r1", [N_IMG, NCW], FP32, kind="ExternalOutput").ap()
        d_dbg_r2 = nc.dram_tensor("dbg_r2", [N_IMG, NCW], FP32, kind="ExternalOutput").ap()
        d_dbg_mT = nc.dram_tensor("dbg_mT", [N_IMG, NCW], FP32, kind="ExternalOutput").ap()
        d_dbg_e = nc.dram_tensor("dbg_e", [128, NCW], FP32, kind="ExternalOutput").ap()
        d_dbg_ss = nc.dram_tensor("dbg_ss", [N_IMG, NCW], FP32, kind="ExternalOutput").ap()
        d_dbg_sa = nc.dram_tensor("dbg_sa", [N_IMG, NCW], FP32, kind="ExternalOutput").ap()
        d_dbg_sb = nc.dram_tensor("dbg_sb", [N_IMG, NCW], FP32, kind="ExternalOutput").ap()

    with tile.TileContext(nc) as tc, ExitStack() as ctx:
        const = ctx.enter_context(tc.tile_pool(name="const", bufs=1))
        # resident inputs
        imgsT = [const.tile([128, NR], FP32R, tag=f"imgsT{k}", name=f"imgsT{k}")
                 for k in range(KCHUNKS)]
        capsT = [const.tile([128, NCW], FP32R, tag=f"capsT{k}", name=f"capsT{k}")
                 for k in range(KCHUNKS)]
        imgsL = [const.tile([128, NR], FP32R, tag=f"imgsL{k}", name=f"imgsL{k}")
                 for k in range(KCHUNKS)]
        capsL = [const.tile([128, NCW], FP32R, tag=f"capsL{k}", name=f"capsL{k}")
                 for k in range(KCHUNKS)]
        for k in range(KCHUNKS):
            nc.sync.dma_start(imgsT[k][:], d_imgsT[128 * k:128 * (k + 1), :])
            nc.sync.dma_start(capsT[k][:], d_capsT[128 * k:128 * (k + 1), :])
            nc.sync.dma_start(imgsL[k][:], d_imgsL[128 * k:128 * (k + 1), :])
            nc.sync.dma_start(capsL[k][:], d_capsL[128 * k:128 * (k + 1), :])
        kbd = const.tile([128, kbd_cols], FP32R, tag="kbd")
        nc.sync.dma_start(kbd[:], d_kbd[:])
        esel = const.tile([N_IMG, esel_cols], FP32R, tag="esel")
        nc.sync.dma_start(esel[:], d_esel[:])
        eselb = const.tile([N_IMG, esel_cols], BF16, tag="eselb")
        nc.sync.dma_start(eselb[:], d_eselb[:])
        eselnb = const.tile([N_IMG, esel_cols], BF16, tag="eselnb")
        nc.sync.dma_start(eselnb[:], d_eselnb[:])
        onesbd = const.tile([128, ones_cols], FP32R, tag="ones")
        nc.sync.dma_start(onesbd[:], d_ones[:])
        ident = const.tile([128, 128], FP32, tag="ident")
        nc.sync.dma_start(ident[:], d_ident[:])
        pbias = const.tile([128, max(1, len(groups))], FP32, tag="pbias")
        nc.sync.dma_start(pbias[:], d_pbias[:])

        stat = ctx.enter_context(tc.tile_pool(name="stat", bufs=1))
        m_T = stat.tile([N_IMG, NCW], FP32, tag="m_T")          # = m_bf, fp32 view
        mT_bf = stat.tile([N_IMG, NCW], BF16, tag="mT_bf")      # bf16(m)
        nt_bf = stat.tile([N_IMG, NCW], BF16, tag="nt_bf")      # bf16(-t')

        phaseA = ExitStack()
        pool_sA = phaseA.enter_context(
            tc.tile_pool(name="psumA", bufs=len(pchunks) + 1, space="PSUM"))
        pool_tp = phaseA.enter_context(tc.tile_pool(name="psumT", bufs=2, space="PSUM"))
        pool_sb = phaseA.enter_context(tc.tile_pool(name="sbA", bufs=2))

        # ---------------- Phase A ----------------
        for mt in range(n_mt):
            lo, hi = mt_bounds[mt]
            mw = hi - lo
            psums = []
            for (ps, pe_) in pchunks:
                cs, ce = offs[ps], offs[pe_]
                p = pool_sA.tile([128, 512], FP32, tag="pA")
                psums.append((p, cs, ce))
                for k in range(KCHUNKS):
                    nc.tensor.matmul(
                        p[:mw, : ce - cs],
                        capsT[k][:, lo:hi],
                        imgsT[k][:, cs:ce],
                        start=(k == 0), stop=(k == KCHUNKS - 1),
                    )
            S = pool_sb.tile([128, NR], FP32, tag="S")
            for j, (p, cs, ce) in enumerate(psums):
                eng = nc.vector if j % 2 == 0 else nc.scalar
                if eng is nc.vector:
                    nc.vector.tensor_copy(S[:mw, cs:ce], p[:mw, : ce - cs])
                else:
                    nc.scalar.activation(S[:mw, cs:ce], p[:mw, : ce - cs], ACTF.Copy)
            mx = pool_sb.tile([128, N_IMG * 8], FP32, tag="mx")
            for i in range(N_IMG):
                nc.vector.max(mx[:mw, 8 * i:8 * i + 8],
                              S[:mw, offs[i]:offs[i] + lens[i]])
            if debug_dump and mt == 0:
                nc.sync.dma_start(d_dbg_S[:mw, :], S[:mw, :])
                nc.sync.dma_start(d_dbg_mx[:mw, :], mx[:mw, :])
            mx3 = mx[:mw, :].rearrange("p (i k) -> p i k", k=8)
            cmp = pool_sb.tile([128, 3 * N_IMG], FP32, tag="cmp")
            c3 = cmp[:mw, :].rearrange("p (j i) -> p j i", j=3)
            m_v = mx3[:, :, 4:5].rearrange("p i k -> p (i k)")
            m5 = mx3[:, :, 5:6].rearrange("p i k -> p (i k)")
            m0 = mx3[:, :, 0:1].rearrange("p i k -> p (i k)")
            t2 = c3[:, 0, :]
            # t2 = m4 + m5  (sum of 5th and 6th largest)
            nc.vector.tensor_add(t2, m_v, m5)
            # rhs1 = -0.5*t2 = -t'
            nc.vector.tensor_scalar(c3[:, 1, :], t2, -0.5, None, op0=ALU.mult)
            # transpose (mw, 64) compacts -> (64, mw); evac casts to bf16 so
            # the folded value and the added-back value are bit-identical
            # (bf16 passes through the PE's fp32r rounding unchanged).
            for src, dst in ((m0, mT_bf), (c3[:, 1, :], nt_bf)):
                pt = pool_tp.tile([N_IMG, 128], FP32, tag="pT")
                nc.tensor.transpose(pt[:, :mw], src, ident[:mw, :mw])
                nc.vector.tensor_copy(dst[:, lo:hi], pt[:, :mw])
            nc.vector.tensor_copy(m_T[:, lo:hi], mT_bf[:, lo:hi])

        if debug_dump:
            nc.sync.dma_start(d_dbg_mT[:], m_T[:])
        phaseA.close()

        # ---------------- Phase B ----------------
        pool_f = ctx.enter_context(tc.tile_pool(name="psumF", bufs=2, space="PSUM"))
        pool_ke = ctx.enter_context(tc.tile_pool(name="psumK", bufs=2, space="PSUM"))
        pool_st = ctx.enter_context(tc.tile_pool(name="psumS", bufs=1, space="PSUM"))
        pool_e = ctx.enter_context(tc.tile_pool(name="sbB", bufs=3))

        st_s = pool_st.tile([N_IMG, NCW], FP32, tag="st_s")
        st_a = pool_st.tile([N_IMG, NCW], FP32, tag="st_a")
        st_b = pool_st.tile([N_IMG, NCW], FP32, tag="st_b")
        # zero-fill all 64 stats partitions (and set has_written) before the
        # group accumulation: lhsT = an all-zero (2, 64) esel slice. Images
        # 32/33 are never in group 0, so esel[32:34, 0:64] is zero; partition
        # base 32 is a legal tile_position.
        for st in (st_s, st_a, st_b):
            nc.tensor.matmul(
                st[:N_IMG, :],
                esel[32:34, 0:N_IMG],
                capsT[0][32:34, :],
                start=True, stop=True, skip_group_check=True,
            )

        col = 0
        ocol = 0
        nG = len(groups)
        for g, (gs, ge) in enumerate(groups):
            rs, re = offs[gs], offs[ge]
            gr = re - rs
            Mg = ge  # stats land at partitions [0:ge) (image index base)
            p1 = pool_f.tile([128, NCW], FP32, tag="p1")
            # S in ~fp32 precision via 3-pass 11-bit hi/lo split
            for ki, (wa, wb) in enumerate(
                [(imgsT, capsT), (imgsT, capsL), (imgsL, capsT)]
            ):
                for k in range(KCHUNKS):
                    nc.tensor.matmul(
                        p1[:gr, :],
                        wa[k][:, rs:re],
                        wb[k][:],
                        start=(ki == 0 and k == 0), stop=False,
                    )
            # += -t'   (S - t' complete; close the accumulation group so the
            # mask can read PSUM -- stop is sim-only metadata)
            nc.tensor.matmul(
                p1[:gr, :],
                eselb[:, col:col + gr],
                nt_bf[:],
                start=False, stop=True,
            )
            mask = pool_e.tile([128, NCW], FP32, tag="mask")
            nc.scalar.activation(mask[:gr, :], p1[:gr, :], ACTF.Sigmoid,
                                 scale=BIGSCALE)
            # += +t' then -m  (now p1 = S - m_bf, with the t' fold cancelled
            # exactly: same tensor, negated selector)
            nc.tensor.matmul(
                p1[:gr, :],
                eselnb[:, col:col + gr],
                nt_bf[:],
                start=False, stop=True,
                skip_group_check=True,
            )
            nc.tensor.matmul(
                p1[:gr, :],
                eselnb[:, col:col + gr],
                mT_bf[:],
                start=False, stop=True,
                skip_group_check=True,
            )
            e0 = pool_e.tile([128, NCW], FP32, tag="e0")
            nc.scalar.activation(e0[:gr, :], p1[:gr, :], ACTF.Exp, scale=INV_T,
                                 bias=pbias[:gr, g:g + 1])
            e = pool_e.tile([128, NCW], FP32R, tag="e")
            nc.gpsimd.tensor_mul(e[:gr, :], e0[:gr, :], mask[:gr, :])
            eW = pool_e.tile([128, NCW], FP32R, tag="eW")
            nc.vector.scalar_tensor_tensor(
                eW[:gr, :], p1[:gr, :], 1.0, e[:gr, :],
                op0=ALU.bypass, op1=ALU.mult,
            )
            pk = pool_ke.tile([128, NCW], FP32, tag="pk")
            nc.tensor.matmul(
                pk[:gr, :],
                kbd[:gr, col:col + gr],
                e[:gr, :],
                start=True, stop=True,
            )
            if debug_dump and g == 0:
                nc.sync.dma_start(d_dbg_e[:gr, :], e[:gr, :].bitcast(FP32))
            eK = pool_e.tile([128, NCW], FP32R, tag="eK")
            nc.vector.scalar_tensor_tensor(
                eK[:gr, :], pk[:gr, :], 1.0, e[:gr, :],
                op0=ALU.bypass, op1=ALU.mult,
            )
            for st, rhs in ((st_s, e), (st_a, eW), (st_b, eK)):
                nc.tensor.matmul(
                    st[:Mg, :],
                    onesbd[:gr, ocol:ocol + Mg],
                    rhs[:gr, :],
                    start=False, stop=(g == nG - 1),
                    skip_group_check=True,
                )
            col += gr
            ocol += Mg

        # ---------------- Final ----------------
        fin = ctx.enter_context(tc.tile_pool(name="fin", bufs=1))
        if debug_dump:
            dss = fin.tile([N_IMG, NCW], FP32, tag="dss")
            nc.vector.tensor_copy(dss[:], st_s[:])
            nc.sync.dma_start(d_dbg_ss[:], dss[:])
            dsa = fin.tile([N_IMG, NCW], FP32, tag="dsa")
            nc.vector.tensor_copy(dsa[:], st_a[:])
            nc.sync.dma_start(d_dbg_sa[:], dsa[:])
            dsb = fin.tile([N_IMG, NCW], FP32, tag="dsb")
            nc.vector.tensor_copy(dsb[:], st_b[:])
            nc.sync.dma_start(d_dbg_sb[:], dsb[:])
        bcl = fin.tile([N_IMG, NCW], FP32, tag="bcl")
        nc.vector.tensor_scalar(bcl[:], st_b[:], 0.0, None, op0=ALU.max)
        sqB = fin.tile([N_IMG, NCW], FP32, tag="sqB")
        nc.scalar.activation(sqB[:], bcl[:], ACTF.Sqrt)
        n1a = fin.tile([N_IMG, NCW], FP32, tag="n1a")
        nc.vector.scalar_tensor_tensor(
            n1a[:], st_s[:], 1.0, m_T[:], op0=ALU.bypass, op1=ALU.mult
        )
        n1 = fin.tile([N_IMG, NCW], FP32, tag="n1")
        nc.vector.tensor_add(n1[:], n1a[:], st_a[:])
        d1 = fin.tile([N_IMG, NCW], FP32, tag="d1")
        nc.vector.scalar_tensor_tensor(
            d1[:], st_s[:], EPS, sqB[:], op0=ALU.mult, op1=ALU.add
        )
        rec = fin.tile([N_IMG, NCW], FP32, tag="rec")
        nc.vector.reciprocal(rec[:], d1[:])
        ov = fin.tile([N_IMG, NCW], FP32, tag="ov")
        nc.vector.tensor_mul(ov[:], n1[:], rec[:])
        nc.sync.dma_start(d_out[:], ov[:])

    nc.compile()
    return nc


def _rne11(x):
    """Round fp32 to 11 explicit mantissa bits (fp32r's internal rounding,
    round-to-nearest-even); returns (hi, lo) with x == hi + lo exactly."""
    u = np.ascontiguousarray(x, dtype=np.float32).view(np.uint32)
    hi = ((u + 0x7FF + ((u >> 12) & 1)) & 0xFFFFF000).view(np.float32)
    lo = (x - hi).astype(np.float32)
    return hi, lo


def kernel(imgs, caps, img_lens, cap_lens, _debug_dump=False):
    imgs = np.asarray(imgs, dtype=np.float32)
    caps = np.asarray(caps, dtype=np.float32)
    il = np.asarray(img_lens).astype(np.int64)
    cl = np.asarray(cap_lens).astype(np.int64)
    n_img, R, d = imgs.shape
    n_cap, W, _ = caps.shape

    lens = il.tolist()
    # fp32r matmuls need even N / 8B-aligned dst: pad each image's region
    # count to even in the ragged layout (pad columns are zero).
    lens_p = [l + (l & 1) for l in lens]
    offs = np.concatenate([[0], np.cumsum(lens_p)]).astype(int).tolist()
    NR = offs[-1]

    # ragged image layout (d, NR)
    imgsT = np.zeros((d, NR), dtype=np.float32)
    for i in range(n_img):
        imgsT[:, offs[i]:offs[i] + lens[i]] = imgs[i, :lens[i], :].T

    # phase-A psum img-chunks (cols <= 512) and flip groups (rows <= 128)
    pchunks = _pack(lens_p, 512)
    groups = _pack(lens_p, 128)

    # per-core caption columns
    caps_per = n_cap // N_CORES
    core_cols = []
    for k in range(N_CORES):
        cols = [(c, w) for c in range(caps_per * k, caps_per * (k + 1))
                for w in range(int(cl[c]))]
        core_cols.append(cols)
    NCW = max(len(c) for c in core_cols)
    NCW = max(NCW, 256)  # keep fp32r matmuls at full rate (N >= 256)
    NCW += NCW & 1       # even N for fp32r

    n_mt = max(1, -(-NCW // 128))
    mtw = -(-NCW // n_mt)
    mt_bounds = []
    lo = 0
    while lo < NCW:
        mt_bounds.append((lo, min(lo + mtw, NCW)))
        lo += mtw
    n_mt = len(mt_bounds)

    # block-diagonal Gram / selector / ones tensors (shared by all cores)
    kbd_cols = sum(offs[e] - offs[s] for (s, e) in groups)
    ones_cols = sum(ge for (_, ge) in groups)
    kbd = np.zeros((128, kbd_cols), dtype=np.float32)
    esel = np.zeros((n_img, kbd_cols), dtype=np.float32)
    onesbd = np.zeros((128, ones_cols), dtype=np.float32)
    padbias = np.zeros((128, max(1, len(groups))), dtype=np.float32)
    col = 0
    ocol = 0
    for g, (gs, ge) in enumerate(groups):
        r0 = offs[gs]
        for i in range(gs, ge):
            a = offs[i] - r0
            b = a + lens[i]           # real rows only; pad row stays zero
            X = imgs[i, :lens[i], :]
            kbd[a:b, col + a:col + b] = (X @ X.T).astype(np.float32)
            esel[i, col + a:col + b] = 1.0
            onesbd[a:b, ocol + i] = 1.0
            if lens_p[i] != lens[i]:
                padbias[b, g] = -1e9  # kill the pad row's exp in this group
        col += offs[ge] - r0
        ocol += ge
    ident = np.eye(128, dtype=np.float32)

    nc = _build_program(lens, offs, NR, NCW, pchunks, groups, n_mt, mt_bounds,
                        debug_dump=_debug_dump)

    imgsT_hi, imgsT_lo = _rne11(imgsT)
    eselb = esel.astype(ml_dtypes.bfloat16)
    eselnb = (-esel).astype(ml_dtypes.bfloat16)
    in_maps = []
    for k in range(N_CORES):
        capsT = np.zeros((d, NCW), dtype=np.float32)
        for j, (c, w) in enumerate(core_cols[k]):
            capsT[:, j] = caps[c, w, :]
        capsT_hi, capsT_lo = _rne11(capsT)
        in_maps.append({
            "imgsT": imgsT_hi, "capsT": capsT_hi,
            "imgsL": imgsT_lo, "capsL": capsT_lo,
            "kbd": kbd, "esel": esel, "eselb": eselb, "eselnb": eselnb,
            "onesbd": onesbd, "ident": ident, "padbias": padbias,
        })

    if _debug_dump:
        res = run_bass_kernel_spmd(nc, in_maps[:1], core_ids=[0])
        kernel._dbg = res.results[0]
        kernel._meta = dict(lens=lens, lens_p=lens_p, offs=offs, NCW=NCW,
                            groups=groups, core_cols=core_cols)
        out = np.full((n_img, n_cap, W), MASK_VAL, dtype=np.float32)
        dev = res.results[0]["out"]
        cols = core_cols[0]
        cc = np.array([c for c, _ in cols]); ww = np.array([w for _, w in cols])
        out[:, cc, ww] = dev[:, :len(cols)]
        return out
    res = run_bass_kernel_spmd(nc, in_maps, core_ids=list(range(N_CORES)))

    out = np.full((n_img, n_cap, W), MASK_VAL, dtype=np.float32)
    for k in range(N_CORES):
        dev = res.results[k]["out"]
        cols = core_cols[k]
        if cols:
            cc = np.array([c for c, _ in cols])
            ww = np.array([w for _, w in cols])
            out[:, cc, ww] = dev[:, :len(cols)]
    return out



# revision 16
# speedup vs baseline: 1.2938x; 1.2938x over previous
"""Bass/Trainium2 kernel for nn_CrossSparseAttention (redesigned).

For every (caption c, image i, word w):
    sims[c,i,w,r] = <caps[c,w], imgs[i,r]> / T   (masked by img/cap lengths)
    keep top-5 per row over r, masked softmax p, att = p @ imgs_m,
    out[i,c,w] = <att, caps[c,w]> / (||att|| + EPS), -1 where w >= cap_len.

Design (8 NeuronCores, caption axis sharded, imgs replicated):
  All matmul inputs are bf16 hi/lo splits (x = hi + lo, both bf16); raw dots
  S are computed to ~fp32 accuracy as hh + hl + lh (3 bf16 passes).

  Phase A (rows = (c,w), ragged (i, r) free): S_hh = hi.hi only; per-image
  DVE max8 -> top-8; m = top1, t' = (5th+6th)/2, transposed to compact
  (n_img, NCW) bf16 tiles.

  Phase B (rows = (i,r) groups <= 128, (c,w) free), software-pipelined:
  hh passes first, the top-5 mask is a plain compare of the hh PSUM state
  against the replicated t' (bit-consistent with phase A's S_hh, so the
  selection is exact); then hl+lh+(-m) accumulate and e = exp(10*(S-m))*mask.
  Per-image sums s = sum e, A' = sum e*(S-m), B = e^T K e (host Gram) are
  accumulated into image-indexed PSUM stats via ones/Gram matmuls.

  Final: out = (A' + m*s) * exp(-0.5*ln(B)) on compact (n_img, NCW) tiles.
  (The EPS=-1e-8 term is ~4e-9 relative and dropped; the shift identity
  out = (A'_z + z*s_z)/sqrt(B_z) holds exactly for any per-(i,cw) shift z,
  here z = bf16(m), so no sigmoid/add-back dance is needed.)

  Host scatters valid (c,w) columns into the full output and fills -1.
"""

import numpy as np
import ml_dtypes
from contextlib import ExitStack

import concourse.bass as bass
import concourse.bacc as bacc
import concourse.tile as tile
import concourse.mybir as mybir
from concourse.bass_utils import run_bass_kernel_spmd

FP32 = mybir.dt.float32
FP32R = mybir.dt.float32r
BF16 = mybir.dt.bfloat16
ALU = mybir.AluOpType
ACTF = mybir.ActivationFunctionType

N_CORES = 8
N_IMG = 64
KNN = 5
INV_T = 10.0          # 1 / TEMPERATURE
MASK_VAL = -1.0
KCHUNKS = 4           # 512 = 4 x 128 contraction chunks


def _pack(sizes, cap):
    """Greedy-pack consecutive items with sum(size) <= cap."""
    out = []
    s = 0
    while s < len(sizes):
        e = s
        tot = 0
        while e < len(sizes) and tot + sizes[e] <= cap:
            tot += sizes[e]
            e += 1
        out.append((s, e))
        s = e
    return out


def _build_program(lens, offs, NR, NCW, pchunks, groups, mt_bounds,
                   debug_dump=False):
    nG = len(groups)
    n_mt = len(mt_bounds)
    nc = bacc.Bacc("TRN2", target_bir_lowering=False, debug=False)

    d_imgsH = nc.dram_tensor("imgsH", [512, NR], BF16, kind="ExternalInput").ap()
    d_capsH = nc.dram_tensor("capsH", [512, NCW], BF16, kind="ExternalInput").ap()
    d_imgsL = nc.dram_tensor("imgsL", [512, NR], BF16, kind="ExternalInput").ap()
    d_capsL = nc.dram_tensor("capsL", [512, NCW], BF16, kind="ExternalInput").ap()
    kbd_cols = sum(offs[e] - offs[s] for (s, e) in groups)
    d_kbd = nc.dram_tensor("kbd", [128, kbd_cols], FP32R, kind="ExternalInput").ap()
    d_eselb = nc.dram_tensor("eselb", [N_IMG, kbd_cols], BF16, kind="ExternalInput").ap()
    d_eselnb = nc.dram_tensor("eselnb", [N_IMG, kbd_cols], BF16, kind="ExternalInput").ap()
    d_ones = nc.dram_tensor("onesbd", [128, N_IMG * nG], FP32R, kind="ExternalInput").ap()
    d_ident = nc.dram_tensor("ident", [128, 128], FP32, kind="ExternalInput").ap()
    d_pbias = nc.dram_tensor("padbias", [128, nG], FP32, kind="ExternalInput").ap()
    d_out = nc.dram_tensor("out", [N_IMG, NCW], FP32, kind="ExternalOutput").ap()
    if debug_dump:
        d_dbg_S = nc.dram_tensor("dbg_S", [128, NR], FP32, kind="ExternalOutput").ap()
        d_dbg_mask = nc.dram_tensor("dbg_mask", [128, NCW], FP32, kind="ExternalOutput").ap()
        d_dbg_e = nc.dram_tensor("dbg_e", [128, NCW], FP32, kind="ExternalOutput").ap()
        d_dbg_pT = nc.dram_tensor("dbg_pT", [N_IMG, NCW], FP32, kind="ExternalOutput").ap()
        d_dbg_ss = nc.dram_tensor("dbg_ss", [N_IMG, NCW], FP32, kind="ExternalOutput").ap()
        d_dbg_sa = nc.dram_tensor("dbg_sa", [N_IMG, NCW], FP32, kind="ExternalOutput").ap()
        d_dbg_sb = nc.dram_tensor("dbg_sb", [N_IMG, NCW], FP32, kind="ExternalOutput").ap()

    with tile.TileContext(nc) as tc, ExitStack() as ctx:
        const = ctx.enter_context(tc.tile_pool(name="const", bufs=1))
        imgsH = [const.tile([128, NR], BF16, tag=f"imgsH{k}", name=f"imgsH{k}")
                 for k in range(KCHUNKS)]
        capsH = [const.tile([128, NCW], BF16, tag=f"capsH{k}", name=f"capsH{k}")
                 for k in range(KCHUNKS)]
        imgsL = [const.tile([128, NR], BF16, tag=f"imgsL{k}", name=f"imgsL{k}")
                 for k in range(KCHUNKS)]
        capsL = [const.tile([128, NCW], BF16, tag=f"capsL{k}", name=f"capsL{k}")
                 for k in range(KCHUNKS)]
        # phase A inputs first so its matmuls can start ASAP
        for k in range(KCHUNKS):
            nc.sync.dma_start(imgsH[k][:], d_imgsH[128 * k:128 * (k + 1), :])
            nc.sync.dma_start(capsH[k][:], d_capsH[128 * k:128 * (k + 1), :])
        ident = const.tile([128, 128], FP32, tag="ident")
        nc.sync.dma_start(ident[:], d_ident[:])
        for k in range(KCHUNKS):
            nc.sync.dma_start(imgsL[k][:], d_imgsL[128 * k:128 * (k + 1), :])
            nc.sync.dma_start(capsL[k][:], d_capsL[128 * k:128 * (k + 1), :])
        eselb = const.tile([N_IMG, kbd_cols], BF16, tag="eselb")
        nc.sync.dma_start(eselb[:], d_eselb[:])
        eselnb = const.tile([N_IMG, kbd_cols], BF16, tag="eselnb")
        nc.sync.dma_start(eselnb[:], d_eselnb[:])
        kbd = const.tile([128, kbd_cols], FP32R, tag="kbd")
        nc.sync.dma_start(kbd[:], d_kbd[:])
        onesbd = const.tile([128, N_IMG * nG], FP32R, tag="ones")
        nc.sync.dma_start(onesbd[:], d_ones[:])
        pbias = const.tile([128, nG], FP32, tag="pbias")
        nc.sync.dma_start(pbias[:], d_pbias[:])

        stat = ctx.enter_context(tc.tile_pool(name="stat", bufs=1))
        m_T = stat.tile([N_IMG, NCW], FP32, tag="m_T")       # z = pT - tmb (fp32)
        tmb_bf = stat.tile([N_IMG, NCW], BF16, tag="tmb")    # bf16(t' - m)
        pT_bf = stat.tile([N_IMG, NCW], BF16, tag="pT")      # bf16(+t')

        # phase-B psum pool opened early: hh prefetch overlaps phase A's tail
        pool_f = ctx.enter_context(tc.tile_pool(name="psumF", bufs=3, space="PSUM"))

        phaseA = ExitStack()
        pool_sA = phaseA.enter_context(
            tc.tile_pool(name="psumA", bufs=3, space="PSUM"))
        pool_tp = phaseA.enter_context(tc.tile_pool(name="psumT", bufs=2, space="PSUM"))
        pool_sb = phaseA.enter_context(tc.tile_pool(name="sbA", bufs=3))

        # ---------------- Phase A ----------------
        tsrc = []     # per-mt (t'_src, m_src) APs for deferred transposes

        def emit_transposes(mt):
            lo, hi = mt_bounds[mt]
            mw = hi - lo
            src_t, src_m = tsrc[mt]
            pt = pool_tp.tile([N_IMG, 128], FP32, tag="pT2", name=f"pt{mt}")
            nc.tensor.transpose(pt[:, :mw], src_t, ident[:mw, :mw])
            pm = pool_tp.tile([N_IMG, 128], FP32, tag="pT2", name=f"pm{mt}")
            nc.tensor.transpose(pm[:, :mw], src_m, ident[:mw, :mw])
            nc.vector.tensor_copy(pT_bf[:, lo:hi], pt[:, :mw])
            mtmp = pool_sb.tile([N_IMG, 128], FP32, tag="mtmp", name=f"mtmp{mt}")
            nc.scalar.activation(mtmp[:, :mw], pm[:, :mw], ACTF.Copy)
            # tmb = bf16(t' - m); z = pT_bf - tmb_bf is the exact shift the
            # PSUM folds apply, reproduced in fp32 for the final formula
            nc.vector.tensor_tensor(tmb_bf[:, lo:hi], pt[:, :mw], mtmp[:, :mw],
                                    op=ALU.subtract)
            nc.vector.tensor_tensor(m_T[:, lo:hi], pT_bf[:, lo:hi],
                                    tmb_bf[:, lo:hi], op=ALU.subtract)

        for mt in range(n_mt):
            lo, hi = mt_bounds[mt]
            mw = hi - lo
            psums = []
            for (ps, pe_) in pchunks:
                cs, ce = offs[ps], offs[pe_]
                p = pool_sA.tile([128, 512], FP32, tag="pA")
                psums.append((p, cs, ce))
                for k in range(KCHUNKS):
                    nc.tensor.matmul(
                        p[:mw, : ce - cs],
                        capsH[k][:, lo:hi],
                        imgsH[k][:, cs:ce],
                        start=(k == 0), stop=(k == KCHUNKS - 1),
                    )
            if mt > 0:
                emit_transposes(mt - 1)
            S = pool_sb.tile([128, NR], FP32, tag="S")
            for j, (p, cs, ce) in enumerate(psums):
                if j % 2 == 0:
                    nc.scalar.activation(S[:mw, cs:ce], p[:mw, : ce - cs], ACTF.Copy)
                else:
                    nc.vector.tensor_copy(S[:mw, cs:ce], p[:mw, : ce - cs])
            if debug_dump and mt == 0:
                nc.sync.dma_start(d_dbg_S[:mw, :], S[:mw, :])
            mx = pool_sb.tile([128, N_IMG * 8], FP32, tag="mx")
            for i in range(N_IMG):
                nc.vector.max(mx[:mw, 8 * i:8 * i + 8],
                              S[:mw, offs[i]:offs[i] + lens[i]])
            mx3 = mx[:mw, :].rearrange("p (i k) -> p i k", k=8)
            cmp = pool_sb.tile([128, 2 * N_IMG], FP32, tag="cmp")
            c2 = cmp[:mw, :].rearrange("p (j i) -> p j i", j=2)
            m4 = mx3[:, :, 4:5].rearrange("p i k -> p (i k)")
            m5 = mx3[:, :, 5:6].rearrange("p i k -> p (i k)")
            m0 = mx3[:, :, 0:1].rearrange("p i k -> p (i k)")
            # t' = (m4 + m5) / 2
            nc.vector.tensor_add(c2[:, 0, :], m4, m5)
            nc.vector.tensor_scalar(c2[:, 1, :], c2[:, 0, :], 0.5, None,
                                    op0=ALU.mult)
            tsrc.append((c2[:, 1, :], m0))

        # ---------------- Phase B ----------------
        # prefetch the first two groups' hh passes to fill phase A's DVE tail
        p1 = {}
        gcol = [sum(offs[e] - offs[s] for (s, e) in groups[:g]) for g in range(nG)]

        def emit_hh(g):
            gs, ge = groups[g]
            rs, re = offs[gs], offs[ge]
            gr = re - rs
            p1[g] = pool_f.tile([128, NCW], FP32, tag="p1", name=f"p1_{g}")
            for k in range(KCHUNKS):
                nc.tensor.matmul(
                    p1[g][:gr, :],
                    imgsH[k][:, rs:re],
                    capsH[k][:],
                    start=(k == 0), stop=False,
                )

        def emit_tfold(g):
            # p1 = S_hh - t'  (closes the group so the mask can read PSUM)
            gs, ge = groups[g]
            gr = offs[ge] - offs[gs]
            nc.tensor.matmul(
                p1[g][:gr, :],
                eselnb[:, gcol[g]:gcol[g] + gr],
                pT_bf[:],
                start=False, stop=True,
                skip_group_check=True,
            )

        emit_hh(0)
        emit_hh(1)
        emit_transposes(n_mt - 1)
        emit_tfold(0)
        emit_tfold(1)
        phaseA.close()

        pool_pk = ctx.enter_context(tc.tile_pool(name="psumK", bufs=2, space="PSUM"))
        pool_st = ctx.enter_context(tc.tile_pool(name="psumS", bufs=1, space="PSUM"))
        pool_e = ctx.enter_context(tc.tile_pool(name="sbB", bufs=2))

        st_s = pool_st.tile([N_IMG, NCW], FP32, tag="st_s")
        st_a = pool_st.tile([N_IMG, NCW], FP32, tag="st_a")
        st_b = pool_st.tile([N_IMG, NCW], FP32, tag="st_b")

        ev = {}

        def emit_stats(g):
            gs, ge = groups[g]
            gr = offs[ge] - offs[gs]
            e_, eW_, v2_ = ev[g]
            for st, rhs in ((st_s, e_), (st_a, eW_), (st_b, v2_)):
                nc.tensor.matmul(
                    st[:N_IMG, :],
                    onesbd[:gr, N_IMG * g:N_IMG * (g + 1)],
                    rhs[:gr, :],
                    start=(g == 0), stop=(g == nG - 1),
                    skip_group_check=True,
                )

        for g in range(nG):
            gs, ge = groups[g]
            rs, re = offs[gs], offs[ge]
            gr = re - rs
            col = gcol[g]
            # top-5 mask: p1 holds S_hh - t', bit-consistent with phase A's
            # S_hh, so the selection is our exact top-5
            mask = pool_e.tile([128, NCW], FP32, tag="mask")
            nc.vector.tensor_scalar(mask[:gr, :], p1[g][:gr, :], 0.0, None,
                                    op0=ALU.is_ge)
            # continue the accumulation: + hl + lh, then + (t' - m)
            for (wa, wb) in ((imgsL, capsH), (imgsH, capsL)):
                for k in range(KCHUNKS):
                    nc.tensor.matmul(
                        p1[g][:gr, :],
                        wa[k][:, rs:re],
                        wb[k][:],
                        start=False, stop=False,
                        skip_group_check=True,
                    )
            nc.tensor.matmul(
                p1[g][:gr, :],
                eselb[:, col:col + gr],
                tmb_bf[:],
                start=False, stop=True,
                skip_group_check=True,
            )
            if g + 2 < nG:
                emit_hh(g + 2)
                emit_tfold(g + 2)
            e0 = pool_e.tile([128, NCW], FP32, tag="e0")
            nc.scalar.activation(e0[:gr, :], p1[g][:gr, :], ACTF.Exp,
                                 scale=INV_T, bias=pbias[:gr, g:g + 1])
            e = pool_e.tile([128, NCW], FP32R, tag="e")
            nc.gpsimd.tensor_mul(e[:gr, :], e0[:gr, :], mask[:gr, :])
            eW = pool_e.tile([128, NCW], FP32R, tag="eW")
            nc.vector.scalar_tensor_tensor(
                eW[:gr, :], p1[g][:gr, :], 1.0, e[:gr, :],
                op0=ALU.bypass, op1=ALU.mult,
            )
            if debug_dump and g == 0:
                nc.sync.dma_start(d_dbg_mask[:gr, :], mask[:gr, :])
                nc.sync.dma_start(d_dbg_e[:gr, :], e[:gr, :].bitcast(FP32))
            if g >= 1:
                emit_stats(g - 1)
            pk = pool_pk.tile([128, NCW], FP32, tag="pk")
            nc.tensor.matmul(
                pk[:gr, :],
                kbd[:gr, col:col + gr],
                e[:gr, :],
                start=True, stop=True,
            )
            # v = L^T e  (kbd holds the Cholesky factor L of the per-image
            # Gram); B = sum v^2 per image, so v^2 goes through ACT Square
            v2 = pool_e.tile([128, NCW], FP32R, tag="v2")
            nc.scalar.activation(v2[:gr, :], pk[:gr, :], ACTF.Square)
            ev[g] = (e, eW, v2)
        emit_stats(nG - 1)

        # ---------------- Final ----------------
        fin = ctx.enter_context(tc.tile_pool(name="fin", bufs=1))
        if debug_dump:
            nc.sync.dma_start(d_dbg_pT[:], m_T[:])
            for dn, st in ((d_dbg_ss, st_s), (d_dbg_sa, st_a), (d_dbg_sb, st_b)):
                dt_ = fin.tile([N_IMG, NCW], FP32, tag="dbgc")
                nc.vector.tensor_copy(dt_[:], st[:])
                nc.sync.dma_start(dn[:], dt_[:])
        bcl = fin.tile([N_IMG, NCW], FP32, tag="bcl")
        nc.vector.tensor_scalar(bcl[:], st_b[:], 1e-20, None, op0=ALU.max)
        lnb = fin.tile([N_IMG, NCW], FP32, tag="lnb")
        nc.scalar.activation(lnb[:], bcl[:], ACTF.Ln)
        rsq = fin.tile([N_IMG, NCW], FP32, tag="rsq")
        nc.scalar.activation(rsq[:], lnb[:], ACTF.Exp, scale=-0.5)
        n1a = fin.tile([N_IMG, NCW], FP32, tag="n1a")
        nc.vector.scalar_tensor_tensor(
            n1a[:], st_s[:], 1.0, m_T[:], op0=ALU.bypass, op1=ALU.mult
        )
        n1 = fin.tile([N_IMG, NCW], FP32, tag="n1")
        nc.vector.tensor_add(n1[:], n1a[:], st_a[:])
        ov = fin.tile([N_IMG, NCW], FP32, tag="ov")
        nc.vector.tensor_mul(ov[:], n1[:], rsq[:])
        nc.sync.dma_start(d_out[:], ov[:])

    nc.compile()
    return nc


def kernel(imgs, caps, img_lens, cap_lens, _debug_dump=False):
    imgs = np.asarray(imgs, dtype=np.float32)
    caps = np.asarray(caps, dtype=np.float32)
    il = np.asarray(img_lens).astype(np.int64)
    cl = np.asarray(cap_lens).astype(np.int64)
    n_img, R, d = imgs.shape
    n_cap, W, _ = caps.shape

    lens = il.tolist()
    lens_p = [l + (l & 1) for l in lens]     # even N / 8B-aligned psum dst
    offs = np.concatenate([[0], np.cumsum(lens_p)]).astype(int).tolist()
    NR = offs[-1]

    imgsT = np.zeros((d, NR), dtype=np.float32)
    for i in range(n_img):
        imgsT[:, offs[i]:offs[i] + lens[i]] = imgs[i, :lens[i], :].T

    pchunks = _pack(lens_p, 512)
    groups = _pack(lens_p, 128)
    nG = len(groups)

    caps_per = n_cap // N_CORES
    core_cols = []
    for k in range(N_CORES):
        cols = [(c, w) for c in range(caps_per * k, caps_per * (k + 1))
                for w in range(int(cl[c]))]
        core_cols.append(cols)
    NCW = max(len(c) for c in core_cols)
    NCW = max(NCW, 256)  # keep fp32r matmuls (kbd/stats) at full rate
    NCW += NCW & 1

    mt_bounds = []
    lo = 0
    while lo < NCW:
        mt_bounds.append((lo, min(lo + 128, NCW)))
        lo += 128

    kbd_cols = sum(offs[e] - offs[s] for (s, e) in groups)
    kbd = np.zeros((128, kbd_cols), dtype=np.float32)
    esel = np.zeros((n_img, kbd_cols), dtype=np.float32)
    onesbd = np.zeros((128, n_img * nG), dtype=np.float32)
    padbias = np.zeros((128, nG), dtype=np.float32)
    col = 0
    for g, (gs, ge) in enumerate(groups):
        r0 = offs[gs]
        for i in range(gs, ge):
            a = offs[i] - r0
            b = a + lens[i]
            X = imgs[i, :lens[i], :].astype(np.float64)
            G = X @ X.T
            L = np.linalg.cholesky(G + 1e-6 * np.eye(lens[i]))
            kbd[a:b, col + a:col + b] = L.astype(np.float32)
            esel[i, col + a:col + b] = 1.0
            onesbd[a:b, n_img * g + i] = 1.0
            if lens_p[i] != lens[i]:
                padbias[b, g] = -1e9  # kill the pad row's exp in this group
        col += offs[ge] - r0
    ident = np.eye(128, dtype=np.float32)

    nc = _build_program(lens, offs, NR, NCW, pchunks, groups, mt_bounds,
                        debug_dump=_debug_dump)

    BF = ml_dtypes.bfloat16
    imgsT_hi = imgsT.astype(BF)
    imgsT_lo = (imgsT - imgsT_hi.astype(np.float32)).astype(BF)
    eselb = esel.astype(BF)
    eselnb = (-esel).astype(BF)
    in_maps = []
    for k in range(N_CORES):
        capsT = np.zeros((d, NCW), dtype=np.float32)
        for j, (c, w) in enumerate(core_cols[k]):
            capsT[:, j] = caps[c, w, :]
        capsT_hi = capsT.astype(BF)
        capsT_lo = (capsT - capsT_hi.astype(np.float32)).astype(BF)
        in_maps.append({
            "imgsH": imgsT_hi, "capsH": capsT_hi,
            "imgsL": imgsT_lo, "capsL": capsT_lo,
            "kbd": kbd, "eselb": eselb, "eselnb": eselnb,
            "onesbd": onesbd, "ident": ident, "padbias": padbias,
        })

    if _debug_dump:
        res = run_bass_kernel_spmd(nc, in_maps[:1], core_ids=[0])
        kernel._dbg = res.results[0]
        kernel._meta = dict(lens=lens, lens_p=lens_p, offs=offs, NCW=NCW,
                            groups=groups, core_cols=core_cols)
        out = np.full((n_img, n_cap, W), MASK_VAL, dtype=np.float32)
        dev = res.results[0]["out"]
        cols = core_cols[0]
        cc = np.array([c for c, _ in cols]); ww = np.array([w for _, w in cols])
        out[:, cc, ww] = dev[:, :len(cols)]
        return out
    res = run_bass_kernel_spmd(nc, in_maps, core_ids=list(range(N_CORES)))

    out = np.full((n_img, n_cap, W), MASK_VAL, dtype=np.float32)
    for k in range(N_CORES):
        dev = res.results[k]["out"]
        cols = core_cols[k]
        if cols:
            cc = np.array([c for c, _ in cols])
            ww = np.array([w for _, w in cols])
            out[:, cc, ww] = dev[:, :len(cols)]
    return out


# revision 26
# speedup vs baseline: 1.5102x; 1.1673x over previous
"""Bass/Trainium2 kernel for nn_CrossSparseAttention (redesigned).

For every (caption c, image i, word w):
    sims[c,i,w,r] = <caps[c,w], imgs[i,r]> / T   (masked by img/cap lengths)
    keep top-5 per row over r, masked softmax p, att = p @ imgs_m,
    out[i,c,w] = <att, caps[c,w]> / (||att|| + EPS), -1 where w >= cap_len.

Design (8 NeuronCores, caption axis sharded, imgs replicated):
  All matmul inputs are bf16 hi/lo splits (x = hi + lo, both bf16); raw dots
  S are computed to ~fp32 accuracy as hh + hl + lh (3 bf16 passes).

  Phase A (rows = (c,w), ragged (i, r) free): S_hh = hi.hi only; per-image
  DVE max8 -> top-8; m = top1, t' = (5th+6th)/2, transposed to compact
  (n_img, NCW) bf16 tiles.

  Phase B (rows = (i,r) groups <= 128, (c,w) free), software-pipelined:
  hh passes first, the top-5 mask is a plain compare of the hh PSUM state
  against the replicated t' (bit-consistent with phase A's S_hh, so the
  selection is exact); then hl+lh+(-m) accumulate and e = exp(10*(S-m))*mask.
  Per-image sums s = sum e, A' = sum e*(S-m), B = e^T K e (host Gram) are
  accumulated into image-indexed PSUM stats via ones/Gram matmuls.

  Final: out = (A' + m*s) * exp(-0.5*ln(B)) on compact (n_img, NCW) tiles.
  (The EPS=-1e-8 term is ~4e-9 relative and dropped; the shift identity
  out = (A'_z + z*s_z)/sqrt(B_z) holds exactly for any per-(i,cw) shift z,
  here z = bf16(m), so no sigmoid/add-back dance is needed.)

  Host scatters valid (c,w) columns into the full output and fills -1.
"""

import numpy as np
import ml_dtypes
from contextlib import ExitStack

import concourse.bass as bass
import concourse.bacc as bacc
import concourse.tile as tile
import concourse.mybir as mybir
from concourse.bass_utils import run_bass_kernel_spmd

FP32 = mybir.dt.float32
FP32R = mybir.dt.float32r
BF16 = mybir.dt.bfloat16
ALU = mybir.AluOpType
ACTF = mybir.ActivationFunctionType

N_CORES = 8
N_IMG = 64
KNN = 5
INV_T = 10.0          # 1 / TEMPERATURE
MASK_VAL = -1.0
KCHUNKS = 4           # 512 = 4 x 128 contraction chunks


def _pack(sizes, cap):
    """Greedy-pack consecutive items with sum(size) <= cap."""
    out = []
    s = 0
    while s < len(sizes):
        e = s
        tot = 0
        while e < len(sizes) and tot + sizes[e] <= cap:
            tot += sizes[e]
            e += 1
        out.append((s, e))
        s = e
    return out


def _build_program(lens, offs, NR, NCW, pchunks, groups, mt_bounds,
                   debug_dump=False):
    nG = len(groups)
    n_mt = len(mt_bounds)
    nc = bacc.Bacc("TRN2", target_bir_lowering=False, debug=False)

    d_imgsH = nc.dram_tensor("imgsH", [512, NR], BF16, kind="ExternalInput").ap()
    d_capsH = nc.dram_tensor("capsH", [512, NCW], BF16, kind="ExternalInput").ap()
    d_imgsL = nc.dram_tensor("imgsL", [512, NR], BF16, kind="ExternalInput").ap()
    d_capsL = nc.dram_tensor("capsL", [512, NCW], BF16, kind="ExternalInput").ap()
    kbd_cols = sum(offs[e] - offs[s] for (s, e) in groups)
    d_kbd = nc.dram_tensor("kbd", [128, kbd_cols], FP32R, kind="ExternalInput").ap()
    d_eselb = nc.dram_tensor("eselb", [N_IMG, kbd_cols], BF16, kind="ExternalInput").ap()
    d_eselnb = nc.dram_tensor("eselnb", [N_IMG, kbd_cols], BF16, kind="ExternalInput").ap()
    d_ones = nc.dram_tensor("onesbd", [128, N_IMG * nG], FP32R, kind="ExternalInput").ap()
    d_ident = nc.dram_tensor("ident", [128, 128], FP32, kind="ExternalInput").ap()
    d_pbias = nc.dram_tensor("padbias", [128, nG], FP32, kind="ExternalInput").ap()
    d_out = nc.dram_tensor("out", [N_IMG, NCW], FP32, kind="ExternalOutput").ap()
    if debug_dump:
        d_dbg_S = nc.dram_tensor("dbg_S", [128, NR], FP32, kind="ExternalOutput").ap()
        d_dbg_mask = nc.dram_tensor("dbg_mask", [128, NCW], FP32, kind="ExternalOutput").ap()
        d_dbg_e = nc.dram_tensor("dbg_e", [128, NCW], FP32, kind="ExternalOutput").ap()
        d_dbg_pT = nc.dram_tensor("dbg_pT", [N_IMG, NCW], FP32, kind="ExternalOutput").ap()
        d_dbg_ss = nc.dram_tensor("dbg_ss", [N_IMG, NCW], FP32, kind="ExternalOutput").ap()
        d_dbg_sa = nc.dram_tensor("dbg_sa", [N_IMG, NCW], FP32, kind="ExternalOutput").ap()
        d_dbg_sb = nc.dram_tensor("dbg_sb", [N_IMG, NCW], FP32, kind="ExternalOutput").ap()

    with tile.TileContext(nc) as tc, ExitStack() as ctx:
        const = ctx.enter_context(tc.tile_pool(name="const", bufs=1))
        imgsH = [const.tile([128, NR], BF16, tag=f"imgsH{k}", name=f"imgsH{k}")
                 for k in range(KCHUNKS)]
        capsH = [const.tile([128, NCW], BF16, tag=f"capsH{k}", name=f"capsH{k}")
                 for k in range(KCHUNKS)]
        imgsL = [const.tile([128, NR], BF16, tag=f"imgsL{k}", name=f"imgsL{k}")
                 for k in range(KCHUNKS)]
        capsL = [const.tile([128, NCW], BF16, tag=f"capsL{k}", name=f"capsL{k}")
                 for k in range(KCHUNKS)]
        # phase A inputs first, sliced at pchunk boundaries so the first
        # matmuls can start after ~0.5MB instead of the full image set
        for k in range(KCHUNKS):
            nc.sync.dma_start(capsH[k][:], d_capsH[128 * k:128 * (k + 1), :])
            cs, ce = offs[pchunks[0][0]], offs[pchunks[0][1]]
            nc.sync.dma_start(imgsH[k][:, cs:ce],
                              d_imgsH[128 * k:128 * (k + 1), cs:ce])
        for (ps, pe_) in pchunks[1:]:
            cs, ce = offs[ps], offs[pe_]
            for k in range(KCHUNKS):
                nc.sync.dma_start(imgsH[k][:, cs:ce],
                                  d_imgsH[128 * k:128 * (k + 1), cs:ce])
        ident = const.tile([128, 128], FP32, tag="ident")
        nc.sync.dma_start(ident[:], d_ident[:])
        for k in range(KCHUNKS):
            nc.sync.dma_start(imgsL[k][:], d_imgsL[128 * k:128 * (k + 1), :])
            nc.sync.dma_start(capsL[k][:], d_capsL[128 * k:128 * (k + 1), :])
        eselb = const.tile([N_IMG, kbd_cols], BF16, tag="eselb")
        nc.sync.dma_start(eselb[:], d_eselb[:])
        eselnb = const.tile([N_IMG, kbd_cols], BF16, tag="eselnb")
        nc.sync.dma_start(eselnb[:], d_eselnb[:])
        kbd = const.tile([128, kbd_cols], FP32R, tag="kbd")
        nc.sync.dma_start(kbd[:], d_kbd[:])
        onesbd = const.tile([128, N_IMG * nG], FP32R, tag="ones")
        nc.sync.dma_start(onesbd[:], d_ones[:])
        pbias = const.tile([128, nG], FP32, tag="pbias")
        nc.sync.dma_start(pbias[:], d_pbias[:])

        stat = ctx.enter_context(tc.tile_pool(name="stat", bufs=1))
        m_T = stat.tile([N_IMG, NCW], FP32, tag="m_T")       # z = pT - tmb (fp32)
        tmb_bf = stat.tile([N_IMG, NCW], BF16, tag="tmb")    # bf16(t' - m)
        pT_bf = stat.tile([N_IMG, NCW], BF16, tag="pT")      # bf16(+t')

        # phase-B psum pool opened early: hh prefetch overlaps phase A's tail
        pool_f = ctx.enter_context(tc.tile_pool(name="psumF", bufs=4, space="PSUM"))

        phaseA = ExitStack()
        pool_sA = phaseA.enter_context(
            tc.tile_pool(name="psumA", bufs=2, space="PSUM"))
        pool_tp = phaseA.enter_context(tc.tile_pool(name="psumT", bufs=2, space="PSUM"))
        pool_sb = phaseA.enter_context(tc.tile_pool(name="sbA", bufs=3))

        # ---------------- Phase A ----------------
        tsrc = []     # per-mt (t'_src, m_src) APs for deferred transposes

        def emit_transposes(mt):
            lo, hi = mt_bounds[mt]
            mw = hi - lo
            src_t, src_m = tsrc[mt]
            pt = pool_tp.tile([N_IMG, 128], FP32, tag="pT2", name=f"pt{mt}")
            nc.tensor.transpose(pt[:, :mw], src_t, ident[:mw, :mw])
            pm = pool_tp.tile([N_IMG, 128], FP32, tag="pT2", name=f"pm{mt}")
            nc.tensor.transpose(pm[:, :mw], src_m, ident[:mw, :mw])
            nc.vector.tensor_copy(pT_bf[:, lo:hi], pt[:, :mw])
            mtmp = pool_sb.tile([N_IMG, 128], FP32, tag="mtmp", name=f"mtmp{mt}")
            nc.scalar.activation(mtmp[:, :mw], pm[:, :mw], ACTF.Copy)
            # tmb = bf16(t' - m); z = pT_bf - tmb_bf is the exact shift the
            # PSUM folds apply, reproduced in fp32 for the final formula
            nc.vector.tensor_tensor(tmb_bf[:, lo:hi], pt[:, :mw], mtmp[:, :mw],
                                    op=ALU.subtract)
            nc.gpsimd.tensor_tensor(m_T[:, lo:hi], pT_bf[:, lo:hi],
                                    tmb_bf[:, lo:hi], op=ALU.subtract)

        for mt in range(n_mt):
            lo, hi = mt_bounds[mt]
            mw = hi - lo
            psums = []
            for (ps, pe_) in pchunks:
                cs, ce = offs[ps], offs[pe_]
                p = pool_sA.tile([128, 512], FP32, tag="pA")
                psums.append((p, cs, ce))
                for k in range(KCHUNKS):
                    nc.tensor.matmul(
                        p[:mw, : ce - cs],
                        capsH[k][:, lo:hi],
                        imgsH[k][:, cs:ce],
                        start=(k == 0), stop=(k == KCHUNKS - 1),
                    )
            if mt > 0:
                emit_transposes(mt - 1)
            # evacs on ACT, max8 on DVE, interleaved per pchunk so the DVE
            # starts each pchunk's top-8 as soon as its copy lands
            S = pool_sb.tile([128, NR], FP32, tag="S")
            mx = pool_sb.tile([128, N_IMG * 8], FP32, tag="mx")
            for (p, cs, ce) in psums:
                nc.scalar.activation(S[:mw, cs:ce], p[:mw, : ce - cs], ACTF.Copy)
                for i in range(N_IMG):
                    if cs <= offs[i] < ce:
                        nc.vector.max(mx[:mw, 8 * i:8 * i + 8],
                                      S[:mw, offs[i]:offs[i] + lens[i]])
            if debug_dump and mt == 0:
                nc.sync.dma_start(d_dbg_S[:mw, :], S[:mw, :])
            mx3 = mx[:mw, :].rearrange("p (i k) -> p i k", k=8)
            cmp = pool_sb.tile([128, 2 * N_IMG], FP32, tag="cmp")
            c2 = cmp[:mw, :].rearrange("p (j i) -> p j i", j=2)
            m4 = mx3[:, :, 4:5].rearrange("p i k -> p (i k)")
            m5 = mx3[:, :, 5:6].rearrange("p i k -> p (i k)")
            m0 = mx3[:, :, 0:1].rearrange("p i k -> p (i k)")
            # t' = (m4 + m5) / 2  (on gpsimd: all-SBUF, keeps DVE on max8)
            nc.gpsimd.tensor_add(c2[:, 0, :], m4, m5)
            nc.gpsimd.tensor_scalar(c2[:, 1, :], c2[:, 0, :], 0.5, None,
                                    op0=ALU.mult)
            tsrc.append((c2[:, 1, :], m0))

        # ---------------- Phase B ----------------
        # prefetch the first two groups' hh passes to fill phase A's DVE tail
        p1 = {}
        gcol = [sum(offs[e] - offs[s] for (s, e) in groups[:g]) for g in range(nG)]

        def emit_hh(g):
            gs, ge = groups[g]
            rs, re = offs[gs], offs[ge]
            gr = re - rs
            p1[g] = pool_f.tile([128, NCW], FP32, tag="p1", name=f"p1_{g}")
            for k in range(KCHUNKS):
                nc.tensor.matmul(
                    p1[g][:gr, :],
                    imgsH[k][:, rs:re],
                    capsH[k][:],
                    start=(k == 0), stop=False,
                )

        def emit_tfold(g):
            # p1 = S_hh - t'  (closes the group so the mask can read PSUM)
            gs, ge = groups[g]
            gr = offs[ge] - offs[gs]
            nc.tensor.matmul(
                p1[g][:gr, :],
                eselnb[:, gcol[g]:gcol[g] + gr],
                pT_bf[:],
                start=False, stop=True,
                skip_group_check=True,
            )

        emit_hh(0)
        emit_hh(1)
        emit_hh(2)
        emit_transposes(n_mt - 1)
        emit_tfold(0)
        emit_tfold(1)
        emit_tfold(2)
        phaseA.close()

        pool_pk = ctx.enter_context(tc.tile_pool(name="psumK", bufs=1, space="PSUM"))
        pool_st = ctx.enter_context(tc.tile_pool(name="psumS", bufs=1, space="PSUM"))
        pool_e = ctx.enter_context(tc.tile_pool(name="sbB", bufs=2))

        st_s = pool_st.tile([N_IMG, NCW], FP32, tag="st_s")
        st_a = pool_st.tile([N_IMG, NCW], FP32, tag="st_a")
        st_b = pool_st.tile([N_IMG, NCW], FP32, tag="st_b")

        ev = {}

        def emit_stats(g):
            gs, ge = groups[g]
            gr = offs[ge] - offs[gs]
            e_, eW_, v2_ = ev[g]
            for st, rhs in ((st_s, e_), (st_a, eW_), (st_b, v2_)):
                nc.tensor.matmul(
                    st[:N_IMG, :],
                    onesbd[:gr, N_IMG * g:N_IMG * (g + 1)],
                    rhs[:gr, :],
                    start=(g == 0), stop=(g == nG - 1),
                    skip_group_check=True,
                )

        for g in range(nG):
            gs, ge = groups[g]
            rs, re = offs[gs], offs[ge]
            gr = re - rs
            col = gcol[g]
            # top-5 mask: p1 holds S_hh - t', bit-consistent with phase A's
            # S_hh, so the selection is our exact top-5
            mask = pool_e.tile([128, NCW], FP32, tag="mask")
            nc.vector.tensor_scalar(mask[:gr, :], p1[g][:gr, :], 0.0, None,
                                    op0=ALU.is_ge)
            # continue the accumulation: + hl + lh, then + (t' - m)
            for (wa, wb) in ((imgsL, capsH), (imgsH, capsL)):
                for k in range(KCHUNKS):
                    nc.tensor.matmul(
                        p1[g][:gr, :],
                        wa[k][:, rs:re],
                        wb[k][:],
                        start=False, stop=False,
                        skip_group_check=True,
                    )
            nc.tensor.matmul(
                p1[g][:gr, :],
                eselb[:, col:col + gr],
                tmb_bf[:],
                start=False, stop=True,
                skip_group_check=True,
            )
            if g + 3 < nG:
                emit_hh(g + 3)
            e0 = pool_e.tile([128, NCW], FP32, tag="e0")
            nc.scalar.activation(e0[:gr, :], p1[g][:gr, :], ACTF.Exp,
                                 scale=INV_T, bias=pbias[:gr, g:g + 1])
            e = pool_e.tile([128, NCW], FP32R, tag="e")
            nc.vector.tensor_mul(e[:gr, :], e0[:gr, :], mask[:gr, :])
            eW = pool_e.tile([128, NCW], FP32R, tag="eW")
            nc.vector.scalar_tensor_tensor(
                eW[:gr, :], p1[g][:gr, :], 1.0, e[:gr, :],
                op0=ALU.bypass, op1=ALU.mult,
            )
            if debug_dump and g == 0:
                nc.sync.dma_start(d_dbg_mask[:gr, :], mask[:gr, :])
                nc.sync.dma_start(d_dbg_e[:gr, :], e[:gr, :].bitcast(FP32))
            if g >= 1:
                emit_stats(g - 1)
            if g + 3 < nG:
                emit_tfold(g + 3)
            pk = pool_pk.tile([128, NCW], FP32, tag="pk")
            nc.tensor.matmul(
                pk[:gr, :],
                kbd[:gr, col:col + gr],
                e[:gr, :],
                start=True, stop=True,
            )
            # v = L^T e  (kbd holds the Cholesky factor L of the per-image
            # Gram); B = sum v^2 per image, so v^2 goes through ACT Square
            v2 = pool_e.tile([128, NCW], FP32R, tag="v2")
            nc.scalar.activation(v2[:gr, :], pk[:gr, :], ACTF.Square)
            ev[g] = (e, eW, v2)
        emit_stats(nG - 1)

        # ---------------- Final ----------------
        fin = ctx.enter_context(tc.tile_pool(name="fin", bufs=1))
        if debug_dump:
            nc.sync.dma_start(d_dbg_pT[:], m_T[:])
            for dn, st in ((d_dbg_ss, st_s), (d_dbg_sa, st_a), (d_dbg_sb, st_b)):
                dt_ = fin.tile([N_IMG, NCW], FP32, tag="dbgc")
                nc.vector.tensor_copy(dt_[:], st[:])
                nc.sync.dma_start(dn[:], dt_[:])
        bcl = fin.tile([N_IMG, NCW], FP32, tag="bcl")
        nc.vector.tensor_scalar(bcl[:], st_b[:], 1e-20, None, op0=ALU.max)
        sqb = fin.tile([N_IMG, NCW], FP32, tag="sqb")
        nc.scalar.activation(sqb[:], bcl[:], ACTF.Sqrt)
        rsq = fin.tile([N_IMG, NCW], FP32, tag="rsq")
        nc.vector.reciprocal(rsq[:], sqb[:])
        n1a = fin.tile([N_IMG, NCW], FP32, tag="n1a")
        nc.vector.scalar_tensor_tensor(
            n1a[:], st_s[:], 1.0, m_T[:], op0=ALU.bypass, op1=ALU.mult
        )
        n1 = fin.tile([N_IMG, NCW], FP32, tag="n1")
        nc.vector.tensor_add(n1[:], n1a[:], st_a[:])
        ov = fin.tile([N_IMG, NCW], FP32, tag="ov")
        nc.vector.tensor_mul(ov[:], n1[:], rsq[:])
        nc.sync.dma_start(d_out[:], ov[:])

    nc.compile()
    return nc


def kernel(imgs, caps, img_lens, cap_lens, _debug_dump=False):
    imgs = np.asarray(imgs, dtype=np.float32)
    caps = np.asarray(caps, dtype=np.float32)
    il = np.asarray(img_lens).astype(np.int64)
    cl = np.asarray(cap_lens).astype(np.int64)
    n_img, R, d = imgs.shape
    n_cap, W, _ = caps.shape

    lens = il.tolist()
    lens_p = [l + (l & 1) for l in lens]     # even N / 8B-aligned psum dst
    offs = np.concatenate([[0], np.cumsum(lens_p)]).astype(int).tolist()
    NR = offs[-1]

    imgsT = np.zeros((d, NR), dtype=np.float32)
    for i in range(n_img):
        imgsT[:, offs[i]:offs[i] + lens[i]] = imgs[i, :lens[i], :].T

    pchunks = _pack(lens_p, 512)
    groups = _pack(lens_p, 128)
    nG = len(groups)

    caps_per = n_cap // N_CORES
    core_cols = []
    for k in range(N_CORES):
        cols = [(c, w) for c in range(caps_per * k, caps_per * (k + 1))
                for w in range(int(cl[c]))]
        core_cols.append(cols)
    NCW = max(len(c) for c in core_cols)
    NCW = max(NCW, 256)  # keep fp32r matmuls (kbd/stats) at full rate
    NCW += NCW & 1

    mt_bounds = []
    lo = 0
    while lo < NCW:
        mt_bounds.append((lo, min(lo + 128, NCW)))
        lo += 128

    kbd_cols = sum(offs[e] - offs[s] for (s, e) in groups)
    kbd = np.zeros((128, kbd_cols), dtype=np.float32)
    esel = np.zeros((n_img, kbd_cols), dtype=np.float32)
    onesbd = np.zeros((128, n_img * nG), dtype=np.float32)
    padbias = np.zeros((128, nG), dtype=np.float32)
    col = 0
    for g, (gs, ge) in enumerate(groups):
        r0 = offs[gs]
        for i in range(gs, ge):
            a = offs[i] - r0
            b = a + lens[i]
            X = imgs[i, :lens[i], :].astype(np.float64)
            G = X @ X.T
            L = np.linalg.cholesky(G + 1e-6 * np.eye(lens[i]))
            kbd[a:b, col + a:col + b] = L.astype(np.float32)
            esel[i, col + a:col + b] = 1.0
            onesbd[a:b, n_img * g + i] = 1.0
            if lens_p[i] != lens[i]:
                padbias[b, g] = -1e9  # kill the pad row's exp in this group
        col += offs[ge] - r0
    ident = np.eye(128, dtype=np.float32)

    nc = _build_program(lens, offs, NR, NCW, pchunks, groups, mt_bounds,
                        debug_dump=_debug_dump)

    BF = ml_dtypes.bfloat16
    imgsT_hi = imgsT.astype(BF)
    imgsT_lo = (imgsT - imgsT_hi.astype(np.float32)).astype(BF)
    eselb = esel.astype(BF)
    eselnb = (-esel).astype(BF)
    in_maps = []
    for k in range(N_CORES):
        capsT = np.zeros((d, NCW), dtype=np.float32)
        for j, (c, w) in enumerate(core_cols[k]):
            capsT[:, j] = caps[c, w, :]
        capsT_hi = capsT.astype(BF)
        capsT_lo = (capsT - capsT_hi.astype(np.float32)).astype(BF)
        in_maps.append({
            "imgsH": imgsT_hi, "capsH": capsT_hi,
            "imgsL": imgsT_lo, "capsL": capsT_lo,
            "kbd": kbd, "eselb": eselb, "eselnb": eselnb,
            "onesbd": onesbd, "ident": ident, "padbias": padbias,
        })

    if _debug_dump:
        res = run_bass_kernel_spmd(nc, in_maps[:1], core_ids=[0])
        kernel._dbg = res.results[0]
        kernel._meta = dict(lens=lens, lens_p=lens_p, offs=offs, NCW=NCW,
                            groups=groups, core_cols=core_cols)
        out = np.full((n_img, n_cap, W), MASK_VAL, dtype=np.float32)
        dev = res.results[0]["out"]
        cols = core_cols[0]
        cc = np.array([c for c, _ in cols]); ww = np.array([w for _, w in cols])
        out[:, cc, ww] = dev[:, :len(cols)]
        return out
    res = run_bass_kernel_spmd(nc, in_maps, core_ids=list(range(N_CORES)))

    out = np.full((n_img, n_cap, W), MASK_VAL, dtype=np.float32)
    for k in range(N_CORES):
        dev = res.results[k]["out"]
        cols = core_cols[k]
        if cols:
            cc = np.array([c for c, _ in cols])
            ww = np.array([w for _, w in cols])
            out[:, cc, ww] = dev[:, :len(cols)]
    return out
